# revision 1
# baseline (speedup 1.0000x reference)
"""HRR self-attention (causal holographic binding) on 8 Trainium2 cores.

Math (per batch b, head h, reference semantics):
    qkv = x @ w_qkv ; q,k,v heads of HD=128
    fq,fk,fv = fft(q|k|v, axis=-1)          (length-128 FFT == matmul with DFT matrix)
    kv   = cumsum(fk*fv, axis=seq)          (causal binding)
    vals = ifft(kv * conj(fq)).real
    out  = vals @ w_out

Implementation notes:
  * FFT/iFFT are 128x128 matmuls (HD == 128 == PE tile).  Real-input FFT is
    conjugate-symmetric, so the full spectrum is packed into 128 partition
    rows:  p=0 -> Re bin0, p=1 -> Re bin64, p=2..64 -> Re bins 1..63,
    p=65..127 -> Im bins 1..63.  This makes every cumsum a contiguous-
    partition tensor_tensor_scan along the free (token) axis.
  * Sharding: core c = 2*b + g handles batch b, heads 4g..4g+3.  Each core
    emits a partial out^T (its 4 heads' contribution); the host sums the
    pair of partials per batch.  No cross-core communication.
  * All matmuls run in fp16 (fp32 PSUM accumulate).  DFT matrices are
    pre-scaled by 1/16 per FFT application to keep intermediates inside
    fp16 range; the inverse matrices and the host-side final scale undo it.
  * Scans keep fp32 state on-engine; data/products/outputs are fp16.
"""

import numpy as np

B, S, D, H = 4, 4096, 1024, 8
HD = 128
NCORES = 8
HPC = H // 2            # heads per core
T = 512                 # token chunk (PSUM bank = 512 fp32)
NT = S // T
KK = D // 128           # contraction tiles for the qkv projection
FS = 16.0               # scale folded into each forward DFT matrix
SV = 16.0               # vals stored as vals/SV
SO = 16.0               # outT stored as out/SO  (host multiplies back)


def _build_consts():
    """Forward packed DFT matrices Gk|Gv|Gsn|Gs0 and inverse A1|A2.

    Packed layout (partition row p): p=0..63 -> Re bins 0..63,
    p=64 -> Re bin 64 (Nyquist), p=65..127 -> Im bins 1..63.

    Walrus requires equal partition bases for the two SBUF inputs of a
    DVE op, so the binding products are built from *pre-swapped* spectra
    (fks = Gs0.T k puts Im content in rows 0..63) and every scan reads
    two operands at the same base:
        re-scan rows [0:64):   cumsum(fk*fv [0:64] - fks*fvs [0:64])
        im-scan rows [64:128): cumsum(fk*fvs [64:] + fks*fv [64:])
    The Nyquist bin rides in row 64 of the im-scan: Gk/Gsn carry (-1)^a
    in col 64 while Gv/Gs0 zero it, so (fk*fvs)[64] = fk64*fv64 and
    (fks*fv)[64] = 0.
    """
    n = HD
    a = np.arange(n)
    cos_aj = np.cos(2 * np.pi * np.outer(a, np.arange(64)) / n)   # [a, j]
    sin_aj = np.sin(2 * np.pi * np.outer(a, np.arange(64)) / n)
    nyq = np.where(a % 2 == 0, 1.0, -1.0)              # (-1)^a

    def fwd(re_cols, col64, im_cols):
        M = np.zeros((n, n))
        M[:, :64] = re_cols
        M[:, 64] = col64
        M[:, 65:] = im_cols[:, 1:]                     # im bins 1..63
        return M

    Gk = fwd(cos_aj, nyq, -sin_aj)                     # fq uses Gk too
    Gv = fwd(cos_aj, 0.0, -sin_aj)
    Gsn = fwd(-sin_aj, nyq, cos_aj)                    # swapped, Nyquist col (for v)
    Gs0 = fwd(-sin_aj, 0.0, cos_aj)                    # swapped, zero col (for k, q)

    # inverse: vals_n = sum_p A1[p,n] P1[p] + A2[p,n] P2[p]
    cos_jn = np.cos(2 * np.pi * np.outer(np.arange(64), a) / n)   # [j, n]
    sin_jn = np.sin(2 * np.pi * np.outer(np.arange(64), a) / n)
    w = np.full(64, 2.0)
    w[0] = 1.0
    A1 = np.zeros((n, n))
    A1[:64, :] = w[:, None] * cos_jn / n
    A1[64, :] = np.where(np.arange(n) % 2 == 0, 1.0, -1.0) / n    # Nyquist (-1)^n
    A1[65:, :] = 2.0 * cos_jn[1:] / n
    A2 = np.zeros((n, n))
    A2[:64, :] = 2.0 * sin_jn / n
    A2[64, :] = 0.0
    A2[65:, :] = -2.0 * sin_jn[1:] / n

    Amul = FS ** 3 / SV
    gmat = np.concatenate(
        [Gk / FS, Gv / FS, Gsn / FS, Gs0 / FS], axis=1).astype(np.float16)  # [128, 512]
    amat = np.concatenate([A1 * Amul, A2 * Amul], axis=1).astype(np.float16)  # [128, 256]
    return gmat, amat


def _build_program():
    import concourse.bass as bass
    import concourse.bacc as bacc
    import concourse.mybir as mybir
    import concourse.tile as tile

    f16 = mybir.dt.float16
    f32 = mybir.dt.float32
    add = mybir.AluOpType.add
    sub = mybir.AluOpType.subtract

    nc = bacc.Bacc("TRN2", target_bir_lowering=False, debug=False)
    xT = nc.dram_tensor("xT", [D, S], f16, kind="ExternalInput").ap()
    wq = nc.dram_tensor("wq", [D, 3 * HPC * 128], f16, kind="ExternalInput").ap()
    wo = nc.dram_tensor("wo", [HPC * 128, D], f16, kind="ExternalInput").ap()
    gmat = nc.dram_tensor("gmat", [128, 512], f16, kind="ExternalInput").ap()
    amat = nc.dram_tensor("amat", [128, 256], f16, kind="ExternalInput").ap()
    outT = nc.dram_tensor("outT", [D, S], f16, kind="ExternalOutput").ap()

    with tile.TileContext(nc) as tc:
        with (
            tc.tile_pool(name="consts", bufs=1) as cpool,
            tc.tile_pool(name="xin", bufs=2) as xpool,
            tc.tile_pool(name="work", bufs=2) as wpool,
            tc.tile_pool(name="kvp", bufs=2) as kvpool,
            tc.tile_pool(name="psA", bufs=1, space="PSUM") as psA,
            tc.tile_pool(name="psB", bufs=1, space="PSUM") as psB,
            tc.tile_pool(name="psC", bufs=1, space="PSUM") as psC,
        ):
            wq_sb = []
            for k in range(KK):
                wqt = cpool.tile([128, 3 * HPC * 128], f16, name=f"wq{k}")
                nc.sync.dma_start(out=wqt, in_=wq[k * 128:(k + 1) * 128, :])
                wq_sb.append(wqt)
            wo_sb = []
            for h in range(HPC):
                wot = cpool.tile([128, D], f16, name=f"wo{h}")
                nc.sync.dma_start(out=wot, in_=wo[h * 128:(h + 1) * 128, :])
                wo_sb.append(wot)
            g_sb = cpool.tile([128, 512], f16, name="g_sb")
            nc.sync.dma_start(out=g_sb, in_=gmat)
            a_sb = cpool.tile([128, 256], f16, name="a_sb")
            nc.sync.dma_start(out=a_sb, in_=amat)

            kv_prev = [None] * HPC
            for t in range(NT):
                ts = slice(t * T, (t + 1) * T)
                xk = []
                for k in range(KK):
                    xkt = xpool.tile([128, T], f16, tag=f"xk{k}", name=f"x_{t}_{k}")
                    nc.sync.dma_start(out=xkt, in_=xT[k * 128:(k + 1) * 128, ts])
                    xk.append(xkt)
                vals_sb = []
                for h in range(HPC):
                    # projection q|k|v for this head
                    ps_qkv = psA.tile([128, 3 * T], f32, tag="A", name=f"psqkv_{t}_{h}")
                    for comp in range(3):
                        col0 = (h * 3 + comp) * 128
                        for k in range(KK):
                            nc.tensor.matmul(
                                ps_qkv[:, comp * T:(comp + 1) * T],
                                lhsT=wq_sb[k][:, col0:col0 + 128],
                                rhs=xk[k],
                                start=(k == 0),
                                stop=(k == KK - 1),
                            )
                    qkv_sb = wpool.tile([128, 3 * T], f16, tag="qkv", name=f"qkv_{t}_{h}")
                    nc.vector.tensor_copy(qkv_sb, ps_qkv)
                    qs = qkv_sb[:, 0:T]
                    ks = qkv_sb[:, T:2 * T]
                    vs = qkv_sb[:, 2 * T:3 * T]
                    # packed FFTs
                    ps_fkv = psB.tile([128, 4 * T], f32, tag="B", name=f"psfkv_{t}_{h}")
                    nc.tensor.matmul(ps_fkv[:, 0:T], lhsT=g_sb[:, 0:128], rhs=ks)
                    nc.tensor.matmul(ps_fkv[:, T:2 * T], lhsT=g_sb[:, 384:512], rhs=ks)
                    nc.tensor.matmul(ps_fkv[:, 2 * T:3 * T], lhsT=g_sb[:, 128:256], rhs=vs)
                    nc.tensor.matmul(ps_fkv[:, 3 * T:4 * T], lhsT=g_sb[:, 256:384], rhs=vs)
                    ps_fq2 = psA.tile([128, 2 * T], f32, tag="A", name=f"psfq_{t}_{h}")
                    nc.tensor.matmul(ps_fq2[:, 0:T], lhsT=g_sb[:, 0:128], rhs=qs)
                    nc.tensor.matmul(ps_fq2[:, T:2 * T], lhsT=g_sb[:, 384:512], rhs=qs)
                    fkv_sb = wpool.tile([128, 4 * T], f16, tag="fkv", name=f"fkv_{t}_{h}")
                    nc.scalar.copy(fkv_sb, ps_fkv)
                    fq2_sb = wpool.tile([128, 2 * T], f16, tag="fq2", name=f"fq2_{t}_{h}")
                    nc.scalar.copy(fq2_sb, ps_fq2)
                    fk_s = fkv_sb[:, 0:T]
                    fks_s = fkv_sb[:, T:2 * T]
                    fv_s = fkv_sb[:, 2 * T:3 * T]
                    fvs_s = fkv_sb[:, 3 * T:4 * T]
                    fq_s = fq2_sb[:, 0:T]
                    fqs_s = fq2_sb[:, T:2 * T]
                    # binding products: Pa=fk*fv Pb=fks*fvs Pc=fk*fvs Pd=fks*fv
                    pk = wpool.tile([128, 4 * T], f16, tag="pk", name=f"pk_{t}_{h}")
                    nc.vector.tensor_mul(pk[0:64, 0:T], fk_s[0:64, :], fv_s[0:64, :])
                    nc.vector.tensor_mul(pk[0:64, T:2 * T], fks_s[0:64, :], fvs_s[0:64, :])
                    nc.vector.tensor_mul(pk[64:128, 2 * T:3 * T], fk_s[64:128, :], fvs_s[64:128, :])
                    nc.vector.tensor_mul(pk[64:128, 3 * T:4 * T], fks_s[64:128, :], fv_s[64:128, :])
                    # causal cumsum (carry chained across chunks)
                    kvt = kvpool.tile([128, T], f16, tag=f"kv{h}", name=f"kv_{t}_{h}")
                    if t == 0:
                        init_r = init_i = 0.0
                    else:
                        p = kv_prev[h]
                        init_r = p[0:64, T - 1:T]
                        init_i = p[64:128, T - 1:T]
                    nc.vector.tensor_tensor_scan(
                        kvt[0:64, :], pk[0:64, 0:T], pk[0:64, T:2 * T], init_r, add, sub)
                    nc.vector.tensor_tensor_scan(
                        kvt[64:128, :], pk[64:128, 2 * T:3 * T], pk[64:128, 3 * T:4 * T],
                        init_i, add, add)
                    kv_prev[h] = kvt
                    # unbinding products
                    p12 = wpool.tile([128, 2 * T], f16, tag="p12", name=f"p12_{t}_{h}")
                    nc.vector.tensor_mul(p12[:, 0:T], kvt, fq_s)
                    nc.vector.tensor_mul(p12[:, T:2 * T], kvt, fqs_s)
                    # inverse fft (accumulate the two halves)
                    ps_vals = psC.tile([128, T], f32, tag="C", name=f"psv_{t}_{h}")
                    nc.tensor.matmul(ps_vals, lhsT=a_sb[:, 0:128], rhs=p12[:, 0:T],
                                     start=True, stop=False)
                    nc.tensor.matmul(ps_vals, lhsT=a_sb[:, 128:256], rhs=p12[:, T:2 * T],
                                     start=False, stop=True)
                    vt = wpool.tile([128, T], f16, tag=f"vals{h}", name=f"vals_{t}_{h}")
                    nc.scalar.copy(vt, ps_vals)
                    vals_sb.append(vt)
                # output projection (partial over this core's heads)
                for od in range(D // 128):
                    ps_out = psC.tile([128, T], f32, tag="C", name=f"pso_{t}_{od}")
                    for h in range(HPC):
                        nc.tensor.matmul(ps_out,
                                         lhsT=wo_sb[h][:, od * 128:(od + 1) * 128],
                                         rhs=vals_sb[h],
                                         start=(h == 0), stop=(h == HPC - 1))
                    ot = wpool.tile([128, T], f16, tag="ot", name=f"ot_{t}_{od}")
                    nc.scalar.copy(ot, ps_out)
                    nc.sync.dma_start(out=outT[od * 128:(od + 1) * 128, ts], in_=ot)
    nc.compile()
    return nc


def _make_in_maps(x, w_qkv, w_out):
    gmat, amat = _build_consts()
    x16 = x.astype(np.float16)
    wq16 = w_qkv.astype(np.float16)
    wo16 = (w_out * (SV / SO)).astype(np.float16)
    in_maps = []
    for c in range(NCORES):
        b, g = divmod(c, 2)
        heads = range(4 * g, 4 * g + 4)
        wq_cols = np.concatenate(
            [wq16[:, comp * D + h * 128: comp * D + (h + 1) * 128]
             for h in heads for comp in range(3)], axis=1)
        wo_rows = np.concatenate(
            [wo16[h * 128:(h + 1) * 128, :] for h in heads], axis=0)
        in_maps.append({
            "xT": np.ascontiguousarray(x16[b].T),
            "wq": np.ascontiguousarray(wq_cols),
            "wo": np.ascontiguousarray(wo_rows),
            "gmat": gmat,
            "amat": amat,
        })
    return in_maps


_NC_CACHE = None


def _get_program():
    global _NC_CACHE
    if _NC_CACHE is None:
        _NC_CACHE = _build_program()
    return _NC_CACHE


def kernel(x, w_qkv, w_out, _trace=False, _results_out=None):
    import sys
    if "/opt/trn_rl_repo" not in sys.path:
        sys.path.insert(0, "/opt/trn_rl_repo")
    from concourse.bass_utils import run_bass_kernel_spmd

    x = np.asarray(x)
    w_qkv = np.asarray(w_qkv)
    w_out = np.asarray(w_out)
    nc = _get_program()
    in_maps = _make_in_maps(x, w_qkv, w_out)
    res = run_bass_kernel_spmd(nc, in_maps, list(range(NCORES)), trace=_trace)
    if _results_out is not None:
        _results_out.append(res)
    out = np.empty((B, S, D), np.float32)
    for b in range(B):
        p0 = res.results[2 * b]["outT"].astype(np.float32)
        p1 = res.results[2 * b + 1]["outT"].astype(np.float32)
        out[b] = (p0 + p1).T * SO
    return out



# revision 2
# speedup vs baseline: 1.4023x; 1.4023x over previous
"""HRR self-attention (causal holographic binding) on 8 Trainium2 cores.

Math (per batch b, head h, reference semantics):
    qkv = x @ w_qkv ; q,k,v heads of HD=128
    fq,fk,fv = fft(q|k|v, axis=-1)          (length-128 FFT == matmul with DFT matrix)
    kv   = cumsum(fk*fv, axis=seq)          (causal binding)
    vals = ifft(kv * conj(fq)).real
    out  = vals @ w_out
v2: comp-granular PSUM tiles (1 bank each) + 3/3/2-slot PSUM pools so the
Tile scheduler can software-pipeline across heads/chunks; all PSUM
evacuations on the Scalar engine, DVE reserved for products + scans.

Implementation notes:
  * FFT/iFFT are 128x128 matmuls (HD == 128 == PE tile).  Real-input FFT is
    conjugate-symmetric, so the full spectrum is packed into 128 partition
    rows:  p=0 -> Re bin0, p=64 -> Re bin64 (Nyquist), p=1..63 -> Re bins,
    p=65..127 -> Im bins 1..63.  This makes every cumsum a contiguous-
    partition tensor_tensor_scan along the free (token) axis.
  * Sharding: core c = 2*b + g handles batch b, heads 4g..4g+3.  Each core
    emits a partial out^T (its 4 heads' contribution); the host sums the
    pair of partials per batch.  No cross-core communication.
  * All matmuls run in fp16 (fp32 PSUM accumulate).  DFT matrices are
    pre-scaled by 1/16 per FFT application to keep intermediates inside
    fp16 range; the inverse matrices and the host-side final scale undo it.
  * Scans keep fp32 state on-engine; data/products/outputs are fp16.
"""

import numpy as np

B, S, D, H = 4, 4096, 1024, 8
HD = 128
NCORES = 8
HPC = H // 2            # heads per core
T = 512                 # token chunk (PSUM bank = 512 fp32)
NT = S // T
KK = D // 128           # contraction tiles for the qkv projection
FS = 16.0               # scale folded into each forward DFT matrix
SV = 16.0               # vals stored as vals/SV
SO = 16.0               # outT stored as out/SO  (host multiplies back)


def _build_consts():
    """Forward packed DFT matrices Gk|Gv|Gsn|Gs0 and inverse A1|A2.

    Packed layout (partition row p): p=0..63 -> Re bins 0..63,
    p=64 -> Re bin 64 (Nyquist), p=65..127 -> Im bins 1..63.

    Walrus requires equal partition bases for the two SBUF inputs of a
    DVE op, so the binding products are built from *pre-swapped* spectra
    (fks = Gs0.T k puts Im content in rows 0..63) and every scan reads
    two operands at the same base:
        re-scan rows [0:64):   cumsum(fk*fv [0:64] - fks*fvs [0:64])
        im-scan rows [64:128): cumsum(fk*fvs [64:] + fks*fv [64:])
    The Nyquist bin rides in row 64 of the im-scan: Gk/Gsn carry (-1)^a
    in col 64 while Gv/Gs0 zero it, so (fk*fvs)[64] = fk64*fv64 and
    (fks*fv)[64] = 0.
    """
    n = HD
    a = np.arange(n)
    cos_aj = np.cos(2 * np.pi * np.outer(a, np.arange(64)) / n)   # [a, j]
    sin_aj = np.sin(2 * np.pi * np.outer(a, np.arange(64)) / n)
    nyq = np.where(a % 2 == 0, 1.0, -1.0)              # (-1)^a

    def fwd(re_cols, col64, im_cols):
        M = np.zeros((n, n))
        M[:, :64] = re_cols
        M[:, 64] = col64
        M[:, 65:] = im_cols[:, 1:]                     # im bins 1..63
        return M

    Gk = fwd(cos_aj, nyq, -sin_aj)                     # fq uses Gk too
    Gv = fwd(cos_aj, 0.0, -sin_aj)
    Gsn = fwd(-sin_aj, nyq, cos_aj)                    # swapped, Nyquist col (for v)
    Gs0 = fwd(-sin_aj, 0.0, cos_aj)                    # swapped, zero col (for k, q)

    # inverse: vals_n = sum_p A1[p,n] P1[p] + A2[p,n] P2[p]
    cos_jn = np.cos(2 * np.pi * np.outer(np.arange(64), a) / n)   # [j, n]
    sin_jn = np.sin(2 * np.pi * np.outer(np.arange(64), a) / n)
    w = np.full(64, 2.0)
    w[0] = 1.0
    A1 = np.zeros((n, n))
    A1[:64, :] = w[:, None] * cos_jn / n
    A1[64, :] = np.where(np.arange(n) % 2 == 0, 1.0, -1.0) / n    # Nyquist (-1)^n
    A1[65:, :] = 2.0 * cos_jn[1:] / n
    A2 = np.zeros((n, n))
    A2[:64, :] = 2.0 * sin_jn / n
    A2[64, :] = 0.0
    A2[65:, :] = -2.0 * sin_jn[1:] / n

    Amul = FS ** 3 / SV
    gmat = np.concatenate(
        [Gk / FS, Gv / FS, Gsn / FS, Gs0 / FS], axis=1).astype(np.float16)  # [128, 512]
    amat = np.concatenate([A1 * Amul, A2 * Amul], axis=1).astype(np.float16)  # [128, 256]
    return gmat, amat


def _build_program():
    import concourse.bass as bass
    import concourse.bacc as bacc
    import concourse.mybir as mybir
    import concourse.tile as tile

    f16 = mybir.dt.float16
    f32 = mybir.dt.float32
    add = mybir.AluOpType.add
    sub = mybir.AluOpType.subtract

    nc = bacc.Bacc("TRN2", target_bir_lowering=False, debug=False)
    xT = nc.dram_tensor("xT", [D, S], f16, kind="ExternalInput").ap()
    wq = nc.dram_tensor("wq", [D, 3 * HPC * 128], f16, kind="ExternalInput").ap()
    wo = nc.dram_tensor("wo", [HPC * 128, D], f16, kind="ExternalInput").ap()
    gmat = nc.dram_tensor("gmat", [128, 512], f16, kind="ExternalInput").ap()
    amat = nc.dram_tensor("amat", [128, 256], f16, kind="ExternalInput").ap()
    outT = nc.dram_tensor("outT", [D, S], f16, kind="ExternalOutput").ap()

    with tile.TileContext(nc) as tc:
        with (
            tc.tile_pool(name="consts", bufs=1) as cpool,
            tc.tile_pool(name="xin", bufs=2) as xpool,
            tc.tile_pool(name="work", bufs=2) as wpool,
            tc.tile_pool(name="kvp", bufs=2) as kvpool,
            tc.tile_pool(name="psP", bufs=3, space="PSUM") as psP,
            tc.tile_pool(name="psS", bufs=3, space="PSUM") as psS,
            tc.tile_pool(name="psV", bufs=2, space="PSUM") as psV,
        ):
            # consts: wq k-tiles first (gate the first proj matmuls), then
            # the small DFT matrices (needed ~5us in), wo last (~20us in).
            wq_sb = []
            for k in range(KK):
                wqt = cpool.tile([128, 3 * HPC * 128], f16, name=f"wq{k}")
                nc.sync.dma_start(out=wqt, in_=wq[k * 128:(k + 1) * 128, :])
                wq_sb.append(wqt)
            g_sb = cpool.tile([128, 512], f16, name="g_sb")
            nc.sync.dma_start(out=g_sb, in_=gmat)
            a_sb = cpool.tile([128, 256], f16, name="a_sb")
            nc.sync.dma_start(out=a_sb, in_=amat)
            wo_sb = []
            for h in range(HPC):
                wot = cpool.tile([128, D], f16, name=f"wo{h}")
                nc.sync.dma_start(out=wot, in_=wo[h * 128:(h + 1) * 128, :])
                wo_sb.append(wot)

            kv_prev = [None] * HPC
            for t in range(NT):
                ts = slice(t * T, (t + 1) * T)
                xk = []
                for k in range(KK):
                    xkt = xpool.tile([128, T], f16, tag=f"xk{k}", name=f"x_{t}_{k}")
                    nc.sync.dma_start(out=xkt, in_=xT[k * 128:(k + 1) * 128, ts])
                    xk.append(xkt)
                vals_sb = []
                for h in range(HPC):
                    # projection q|k|v for this head, one PSUM bank per comp
                    comp_sb = []
                    for comp in range(3):
                        ps_c = psP.tile([128, T], f32, tag="p", name=f"psp_{t}_{h}_{comp}")
                        col0 = (h * 3 + comp) * 128
                        for k in range(KK):
                            nc.tensor.matmul(
                                ps_c,
                                lhsT=wq_sb[k][:, col0:col0 + 128],
                                rhs=xk[k],
                                start=(k == 0),
                                stop=(k == KK - 1),
                            )
                        csb = wpool.tile([128, T], f16, tag=f"c{h}_{comp}",
                                         name=f"qkv_{t}_{h}_{comp}")
                        nc.scalar.copy(csb, ps_c)
                        comp_sb.append(csb)
                    qs, ks, vs = comp_sb
                    # packed FFTs: one PSUM bank + one evacuation per spectrum
                    spec = {}
                    for nm, gcol, src in (
                        ("fk", 0, ks), ("fks", 384, ks),
                        ("fv", 128, vs), ("fvs", 256, vs),
                        ("fq", 0, qs), ("fqs", 384, qs),
                    ):
                        ps_f = psS.tile([128, T], f32, tag="s", name=f"psf_{nm}_{t}_{h}")
                        nc.tensor.matmul(ps_f, lhsT=g_sb[:, gcol:gcol + 128], rhs=src)
                        ssb = wpool.tile([128, T], f16, tag=f"{nm}{h}",
                                         name=f"{nm}_{t}_{h}")
                        nc.scalar.copy(ssb, ps_f)
                        spec[nm] = ssb
                    # binding products: Pa=fk*fv Pb=fks*fvs Pc=fk*fvs Pd=fks*fv
                    pk = wpool.tile([128, 4 * T], f16, tag=f"pk{h}", name=f"pk_{t}_{h}")
                    nc.vector.tensor_mul(pk[0:64, 0:T], spec["fk"][0:64, :], spec["fv"][0:64, :])
                    nc.vector.tensor_mul(pk[0:64, T:2 * T], spec["fks"][0:64, :], spec["fvs"][0:64, :])
                    nc.vector.tensor_mul(pk[64:128, 2 * T:3 * T], spec["fk"][64:128, :], spec["fvs"][64:128, :])
                    nc.vector.tensor_mul(pk[64:128, 3 * T:4 * T], spec["fks"][64:128, :], spec["fv"][64:128, :])
                    # causal cumsum (carry chained across chunks)
                    kvt = kvpool.tile([128, T], f16, tag=f"kv{h}", name=f"kv_{t}_{h}")
                    if t == 0:
                        init_r = init_i = 0.0
                    else:
                        p = kv_prev[h]
                        init_r = p[0:64, T - 1:T]
                        init_i = p[64:128, T - 1:T]
                    nc.vector.tensor_tensor_scan(
                        kvt[0:64, :], pk[0:64, 0:T], pk[0:64, T:2 * T], init_r, add, sub)
                    nc.vector.tensor_tensor_scan(
                        kvt[64:128, :], pk[64:128, 2 * T:3 * T], pk[64:128, 3 * T:4 * T],
                        init_i, add, add)
                    kv_prev[h] = kvt
                    # unbinding products
                    p12 = wpool.tile([128, 2 * T], f16, tag=f"p12{h}", name=f"p12_{t}_{h}")
                    nc.vector.tensor_mul(p12[:, 0:T], kvt, spec["fq"])
                    nc.vector.tensor_mul(p12[:, T:2 * T], kvt, spec["fqs"])
                    # inverse fft (accumulate the two halves)
                    ps_vals = psV.tile([128, T], f32, tag="v", name=f"psv_{t}_{h}")
                    nc.tensor.matmul(ps_vals, lhsT=a_sb[:, 0:128], rhs=p12[:, 0:T],
                                     start=True, stop=False)
                    nc.tensor.matmul(ps_vals, lhsT=a_sb[:, 128:256], rhs=p12[:, T:2 * T],
                                     start=False, stop=True)
                    vt = wpool.tile([128, T], f16, tag=f"vals{h}", name=f"vals_{t}_{h}")
                    nc.scalar.copy(vt, ps_vals)
                    vals_sb.append(vt)
                # output projection (partial over this core's heads)
                for od in range(D // 128):
                    ps_out = psV.tile([128, T], f32, tag="v", name=f"pso_{t}_{od}")
                    for h in range(HPC):
                        nc.tensor.matmul(ps_out,
                                         lhsT=wo_sb[h][:, od * 128:(od + 1) * 128],
                                         rhs=vals_sb[h],
                                         start=(h == 0), stop=(h == HPC - 1))
                    ot = wpool.tile([128, T], f16, tag="ot", name=f"ot_{t}_{od}")
                    nc.scalar.copy(ot, ps_out)
                    nc.sync.dma_start(out=outT[od * 128:(od + 1) * 128, ts], in_=ot)
    nc.compile()
    return nc


def _make_in_maps(x, w_qkv, w_out):
    gmat, amat = _build_consts()
    x16 = x.astype(np.float16)
    wq16 = w_qkv.astype(np.float16)
    wo16 = (w_out * (SV / SO)).astype(np.float16)
    in_maps = []
    for c in range(NCORES):
        b, g = divmod(c, 2)
        heads = range(4 * g, 4 * g + 4)
        wq_cols = np.concatenate(
            [wq16[:, comp * D + h * 128: comp * D + (h + 1) * 128]
             for h in heads for comp in range(3)], axis=1)
        wo_rows = np.concatenate(
            [wo16[h * 128:(h + 1) * 128, :] for h in heads], axis=0)
        in_maps.append({
            "xT": np.ascontiguousarray(x16[b].T),
            "wq": np.ascontiguousarray(wq_cols),
            "wo": np.ascontiguousarray(wo_rows),
            "gmat": gmat,
            "amat": amat,
        })
    return in_maps


_NC_CACHE = None


def _get_program():
    global _NC_CACHE
    if _NC_CACHE is None:
        _NC_CACHE = _build_program()
    return _NC_CACHE


def kernel(x, w_qkv, w_out, _trace=False, _results_out=None):
    import sys
    if "/opt/trn_rl_repo" not in sys.path:
        sys.path.insert(0, "/opt/trn_rl_repo")
    from concourse.bass_utils import run_bass_kernel_spmd

    x = np.asarray(x)
    w_qkv = np.asarray(w_qkv)
    w_out = np.asarray(w_out)
    nc = _get_program()
    in_maps = _make_in_maps(x, w_qkv, w_out)
    res = run_bass_kernel_spmd(nc, in_maps, list(range(NCORES)), trace=_trace)
    if _results_out is not None:
        _results_out.append(res)
    out = np.empty((B, S, D), np.float32)
    for b in range(B):
        p0 = res.results[2 * b]["outT"].astype(np.float32)
        p1 = res.results[2 * b + 1]["outT"].astype(np.float32)
        out[b] = (p0 + p1).T * SO
    return out


# revision 3
# speedup vs baseline: 1.5716x; 1.1208x over previous
"""HRR self-attention (causal holographic binding) on 8 Trainium2 cores.

Math (per batch b, head h, reference semantics):
    qkv = x @ w_qkv ; q,k,v heads of HD=128
    fq,fk,fv = fft(q|k|v, axis=-1)          (length-128 FFT == matmul with DFT matrix)
    kv   = cumsum(fk*fv, axis=seq)          (causal binding)
    vals = ifft(kv * conj(fq)).real
    out  = vals @ w_out

v3 pipeline (per head h, token chunk t of 512):
  * Packed real spectrum on 128 partition rows: p=0..63 Re bins 0..63,
    p=64 Nyquist (Re bin 64), p=65..127 Im bins 1..63.
  * 4 forward DFT matmuls only: fk = Gk.T k, fv = Gv.T v, fvs = Gsm.T v,
    fq = Gk.T q, where Gsm = [+sin | nyq | cos] gives
    fvs = [-Im(V) | Nyq(V) | Re(V)].  The old fks/fqs matmuls are gone:
    fqs is an SBUF->SBUF DMA half-swap of fq (its polluted rows 0/64 are
    annihilated by A2's zero rows), and the k-side swap is avoided by
    restructuring the scan inputs:
        in0 = [ fk.Re*fv.Re ; fk.Im*fvs.hi ]   (A ; D)  rows share bases
        W   = [ fk.Re*fvs.lo ; fk.Im*fv.Im ]   (-E ; B)
        in1 = half-swap(W)                      (B ; -E)
    kvt = tensor_tensor_scan(in0, in1, op0=add, op1=subtract)
        -> rows 0:64  cumsum(A - B)  = Re(KV) bins (row0 = DC)
           rows 64:   cumsum(D + E)  = Im(KV) bins (row64 = Nyquist)
    One 128-row scan instead of two 64-row scans.
  * Comp-granular PSUM (1 bank per tile, pools 3/3/2) so the Tile
    scheduler software-pipelines heads and chunks; PSUM evacuations on
    Scalar except ot (DVE); swaps issued from the idle GpSimd queue.
  * Sharding: core c = 2*b + g handles batch b, heads 4g..4g+3; host sums
    the pair of partial outT per batch.  fp16 matmuls, fp32 PSUM; DFT
    matrices pre-scaled by 1/16 per application (undone on host).
"""

import numpy as np

B, S, D, H = 4, 4096, 1024, 8
HD = 128
NCORES = 8
HPC = H // 2            # heads per core
T = 512                 # token chunk (PSUM bank = 512 fp32)
NT = S // T
KK = D // 128           # contraction tiles for the qkv projection
FS = 16.0               # scale folded into each forward DFT matrix
SV = 16.0               # vals stored as vals/SV
SO = 16.0               # outT stored as out/SO  (host multiplies back)


def _build_consts():
    """Forward packed DFT matrices Gk|Gv|Gsm and inverse A1|A2."""
    n = HD
    a = np.arange(n)
    cos_aj = np.cos(2 * np.pi * np.outer(a, np.arange(64)) / n)   # [a, j]
    sin_aj = np.sin(2 * np.pi * np.outer(a, np.arange(64)) / n)
    nyq = np.where(a % 2 == 0, 1.0, -1.0)              # (-1)^a

    def fwd(re_cols, col64, im_cols):
        M = np.zeros((n, n))
        M[:, :64] = re_cols
        M[:, 64] = col64
        M[:, 65:] = im_cols[:, 1:]                     # im bins 1..63
        return M

    Gk = fwd(cos_aj, nyq, -sin_aj)                     # fq uses Gk too
    Gv = fwd(cos_aj, 0.0, -sin_aj)
    Gsm = fwd(+sin_aj, nyq, cos_aj)                    # v-swap, negated lo half

    # inverse: vals_n = sum_p A1[p,n] P1[p] + A2[p,n] P2[p]
    cos_jn = np.cos(2 * np.pi * np.outer(np.arange(64), a) / n)   # [j, n]
    sin_jn = np.sin(2 * np.pi * np.outer(np.arange(64), a) / n)
    w = np.full(64, 2.0)
    w[0] = 1.0
    A1 = np.zeros((n, n))
    A1[:64, :] = w[:, None] * cos_jn / n
    A1[64, :] = np.where(np.arange(n) % 2 == 0, 1.0, -1.0) / n    # Nyquist (-1)^n
    A1[65:, :] = 2.0 * cos_jn[1:] / n
    A2 = np.zeros((n, n))
    A2[:64, :] = 2.0 * sin_jn / n
    A2[64, :] = 0.0
    A2[65:, :] = -2.0 * sin_jn[1:] / n

    Amul = FS ** 3 / SV
    gmat = np.concatenate(
        [Gk / FS, Gv / FS, Gsm / FS], axis=1).astype(np.float16)  # [128, 384]
    amat = np.concatenate([A1 * Amul, A2 * Amul], axis=1).astype(np.float16)  # [128, 256]
    return gmat, amat


def _build_program():
    import concourse.bass as bass
    import concourse.bacc as bacc
    import concourse.mybir as mybir
    import concourse.tile as tile

    f16 = mybir.dt.float16
    f32 = mybir.dt.float32
    add = mybir.AluOpType.add
    sub = mybir.AluOpType.subtract

    nc = bacc.Bacc("TRN2", target_bir_lowering=False, debug=False)
    xT = nc.dram_tensor("xT", [D, S], f16, kind="ExternalInput").ap()
    wq = nc.dram_tensor("wq", [D, 3 * HPC * 128], f16, kind="ExternalInput").ap()
    wo = nc.dram_tensor("wo", [HPC * 128, D], f16, kind="ExternalInput").ap()
    gmat = nc.dram_tensor("gmat", [128, 384], f16, kind="ExternalInput").ap()
    amat = nc.dram_tensor("amat", [128, 256], f16, kind="ExternalInput").ap()
    outT = nc.dram_tensor("outT", [D, S], f16, kind="ExternalOutput").ap()

    with tile.TileContext(nc) as tc:
        with (
            tc.tile_pool(name="consts", bufs=1) as cpool,
            tc.tile_pool(name="xin", bufs=2) as xpool,
            tc.tile_pool(name="work", bufs=2) as wpool,
            tc.tile_pool(name="kvp", bufs=2) as kvpool,
            tc.tile_pool(name="psP", bufs=3, space="PSUM") as psP,
            tc.tile_pool(name="psS", bufs=3, space="PSUM") as psS,
            tc.tile_pool(name="psV", bufs=2, space="PSUM") as psV,
        ):
            # chunk-0 x tiles first: they gate the very first proj matmuls
            xk0 = []
            for k in range(KK):
                xkt = xpool.tile([128, T], f16, tag=f"xk{k}", name=f"x_0_{k}")
                nc.sync.dma_start(out=xkt, in_=xT[k * 128:(k + 1) * 128, 0:T])
                xk0.append(xkt)
            wq_sb = []
            for k in range(KK):
                wqt = cpool.tile([128, 3 * HPC * 128], f16, name=f"wq{k}")
                nc.sync.dma_start(out=wqt, in_=wq[k * 128:(k + 1) * 128, :])
                wq_sb.append(wqt)
            g_sb = cpool.tile([128, 384], f16, name="g_sb")
            nc.sync.dma_start(out=g_sb, in_=gmat)
            a_sb = cpool.tile([128, 256], f16, name="a_sb")
            nc.sync.dma_start(out=a_sb, in_=amat)
            wo_sb = []
            for h in range(HPC):
                wot = cpool.tile([128, D], f16, name=f"wo{h}")
                nc.sync.dma_start(out=wot, in_=wo[h * 128:(h + 1) * 128, :])
                wo_sb.append(wot)

            kv_prev = [None] * HPC
            for t in range(NT):
                ts = slice(t * T, (t + 1) * T)
                if t == 0:
                    xk = xk0
                else:
                    xk = []
                    for k in range(KK):
                        xkt = xpool.tile([128, T], f16, tag=f"xk{k}", name=f"x_{t}_{k}")
                        nc.sync.dma_start(out=xkt, in_=xT[k * 128:(k + 1) * 128, ts])
                        xk.append(xkt)
                vals_sb = []
                for h in range(HPC):
                    # projection q|k|v for this head, one PSUM bank per comp
                    comp_sb = []
                    for comp in range(3):
                        ps_c = psP.tile([128, T], f32, tag="p", name=f"psp_{t}_{h}_{comp}")
                        col0 = (h * 3 + comp) * 128
                        for k in range(KK):
                            nc.tensor.matmul(
                                ps_c,
                                lhsT=wq_sb[k][:, col0:col0 + 128],
                                rhs=xk[k],
                                start=(k == 0),
                                stop=(k == KK - 1),
                            )
                        csb = wpool.tile([128, T], f16, tag=f"c{h}_{comp}",
                                         name=f"qkv_{t}_{h}_{comp}")
                        nc.scalar.copy(csb, ps_c)
                        comp_sb.append(csb)
                    qs, ks, vs = comp_sb
                    # packed FFTs: fk|fq (Gk), fv (Gv), fvs (Gsm)
                    spec = {}
                    for nm, gcol, src in (
                        ("fk", 0, ks), ("fq", 0, qs),
                        ("fv", 128, vs), ("fvs", 256, vs),
                    ):
                        ps_f = psS.tile([128, T], f32, tag="s", name=f"psf_{nm}_{t}_{h}")
                        nc.tensor.matmul(ps_f, lhsT=g_sb[:, gcol:gcol + 128], rhs=src)
                        ssb = wpool.tile([128, T], f16, tag=f"{nm}{h}",
                                         name=f"{nm}_{t}_{h}")
                        nc.scalar.copy(ssb, ps_f)
                        spec[nm] = ssb
                    fk_s, fv_s, fvs_s, fq_s = spec["fk"], spec["fv"], spec["fvs"], spec["fq"]
                    # fqs = half-swap of fq (rows j <-> 64+j); rows 0/64 are
                    # polluted but multiplied by A2's zero rows downstream.
                    fqs = wpool.tile([128, T], f16, tag=f"fqs{h}", name=f"fqs_{t}_{h}")
                    nc.gpsimd.dma_start(out=fqs[0:64, :], in_=fq_s[64:128, :])
                    nc.gpsimd.dma_start(out=fqs[64:128, :], in_=fq_s[0:64, :])
                    # binding products: in0 = [A ; D], W = [-E ; B], in1 = swap(W)
                    in0 = wpool.tile([128, T], f16, tag=f"in0_{h}", name=f"in0_{t}_{h}")
                    nc.vector.tensor_mul(in0[0:64, :], fk_s[0:64, :], fv_s[0:64, :])
                    nc.vector.tensor_mul(in0[64:128, :], fk_s[64:128, :], fvs_s[64:128, :])
                    wt = wpool.tile([128, T], f16, tag=f"wt{h}", name=f"wt_{t}_{h}")
                    nc.vector.tensor_mul(wt[0:64, :], fk_s[0:64, :], fvs_s[0:64, :])
                    nc.vector.tensor_mul(wt[64:128, :], fk_s[64:128, :], fv_s[64:128, :])
                    in1 = wpool.tile([128, T], f16, tag=f"in1_{h}", name=f"in1_{t}_{h}")
                    nc.gpsimd.dma_start(out=in1[0:64, :], in_=wt[64:128, :])
                    nc.gpsimd.dma_start(out=in1[64:128, :], in_=wt[0:64, :])
                    # causal cumsum: state = (in0 + state) - in1, carry-chained
                    kvt = kvpool.tile([128, T], f16, tag=f"kv{h}", name=f"kv_{t}_{h}")
                    init = 0.0 if t == 0 else kv_prev[h][:, T - 1:T]
                    nc.vector.tensor_tensor_scan(kvt, in0, in1, init, add, sub)
                    kv_prev[h] = kvt
                    # unbinding products
                    p12 = wpool.tile([128, 2 * T], f16, tag=f"p12{h}", name=f"p12_{t}_{h}")
                    nc.vector.tensor_mul(p12[:, 0:T], kvt, fq_s)
                    nc.vector.tensor_mul(p12[:, T:2 * T], kvt, fqs)
                    # inverse fft (accumulate the two halves)
                    ps_vals = psV.tile([128, T], f32, tag="v", name=f"psv_{t}_{h}")
                    nc.tensor.matmul(ps_vals, lhsT=a_sb[:, 0:128], rhs=p12[:, 0:T],
                                     start=True, stop=False)
                    nc.tensor.matmul(ps_vals, lhsT=a_sb[:, 128:256], rhs=p12[:, T:2 * T],
                                     start=False, stop=True)
                    vt = wpool.tile([128, T], f16, tag=f"vals{h}", name=f"vals_{t}_{h}")
                    nc.scalar.copy(vt, ps_vals)
                    vals_sb.append(vt)
                # output projection (partial over this core's heads)
                for od in range(D // 128):
                    ps_out = psV.tile([128, T], f32, tag="v", name=f"pso_{t}_{od}")
                    for h in range(HPC):
                        nc.tensor.matmul(ps_out,
                                         lhsT=wo_sb[h][:, od * 128:(od + 1) * 128],
                                         rhs=vals_sb[h],
                                         start=(h == 0), stop=(h == HPC - 1))
                    ot = wpool.tile([128, T], f16, tag="ot", name=f"ot_{t}_{od}")
                    nc.vector.tensor_copy(ot, ps_out)
                    nc.sync.dma_start(out=outT[od * 128:(od + 1) * 128, ts], in_=ot)
    nc.compile()
    return nc


def _make_in_maps(x, w_qkv, w_out):
    gmat, amat = _build_consts()
    x16 = x.astype(np.float16)
    wq16 = w_qkv.astype(np.float16)
    wo16 = (w_out * (SV / SO)).astype(np.float16)
    in_maps = []
    for c in range(NCORES):
        b, g = divmod(c, 2)
        heads = range(4 * g, 4 * g + 4)
        wq_cols = np.concatenate(
            [wq16[:, comp * D + h * 128: comp * D + (h + 1) * 128]
             for h in heads for comp in range(3)], axis=1)
        wo_rows = np.concatenate(
            [wo16[h * 128:(h + 1) * 128, :] for h in heads], axis=0)
        in_maps.append({
            "xT": np.ascontiguousarray(x16[b].T),
            "wq": np.ascontiguousarray(wq_cols),
            "wo": np.ascontiguousarray(wo_rows),
            "gmat": gmat,
            "amat": amat,
        })
    return in_maps


_NC_CACHE = None


def _get_program():
    global _NC_CACHE
    if _NC_CACHE is None:
        _NC_CACHE = _build_program()
    return _NC_CACHE


def kernel(x, w_qkv, w_out, _trace=False, _results_out=None):
    import sys
    if "/opt/trn_rl_repo" not in sys.path:
        sys.path.insert(0, "/opt/trn_rl_repo")
    from concourse.bass_utils import run_bass_kernel_spmd

    x = np.asarray(x)
    w_qkv = np.asarray(w_qkv)
    w_out = np.asarray(w_out)
    nc = _get_program()
    in_maps = _make_in_maps(x, w_qkv, w_out)
    res = run_bass_kernel_spmd(nc, in_maps, list(range(NCORES)), trace=_trace)
    if _results_out is not None:
        _results_out.append(res)
    out = np.empty((B, S, D), np.float32)
    for b in range(B):
        p0 = res.results[2 * b]["outT"].astype(np.float32)
        p1 = res.results[2 * b + 1]["outT"].astype(np.float32)
        out[b] = (p0 + p1).T * SO
    return out
